# revision 2
# baseline (speedup 1.0000x reference)
"""KnnLoss Trainium2 kernel (v15) — single-core, packed-key top-8, fully unrolled.

Math per batch: scores s = 2 q.c - |c|^2 via fp16 matmul (hi/lo split of
|c|^2, host-precomputed). The PSUM->SBUF activation writes bf16 "quality
levels" q = 2040 - S*d2 (scale=S, per-query bias) into the HIGH u16 lanes
of a persistent f32 key tile whose LOW u16 lanes hold the candidate index
(+8192 for batch 1), so each f32 word reads as key = q_bf16 + idx*2^-13.
ONE DVE Max pass then yields top-8 values with indices embedded
(u = key*8192 is an exact integer whose low 16 bits are the global mask
row). Out-of-radius slots (key < THR) are replaced by the self index
arithmetically (no CopyPredicated). The 7 neighbor mask rows are fetched
in ONE merged indirect DMA per tile; L1-diff accumulated on device.

Single core processes both batches (multi-device dispatch through the
axon tunnel costs ~3x a single-device dispatch); the key-tile index lanes
are bumped by +8192 between batches so gathers hit the [2N, KS] table.
"""

import numpy as np

import concourse.bass as bass
import concourse.mybir as mybir
import concourse.tile as tile
from concourse import bacc
from concourse.bass import IndirectOffsetOnAxis, ds, ts
from concourse.bass_utils import run_bass_kernel_spmd

B = 2
N = 8192
KS = 16
KNN = 8

NCORES = 1
UNROLL = 8
NT = N // 128          # 64 tiles per batch
NB = B * N             # mask table rows

F32 = mybir.dt.float32
F16 = mybir.dt.float16
BF16 = mybir.dt.bfloat16
U32 = mybir.dt.uint32
U16 = mybir.dt.uint16

CH = 512
CPY = 2048
NCPY = N // CPY

SCALE = np.float32(79921.875)          # key units per unit d2
BIAS0 = np.float32(2040.0)             # self (d2=0) key level
R2 = np.float32(0.1) * np.float32(0.1)
THR = float(BIAS0 - SCALE * R2)        # in-radius iff key >= THR (+b for batch b)

# blob layout in u32 words
OFF_MASK = 0
W_MASK = NB * KS // 2
OFF_PC = W_MASK                        # per batch: [3, N] f16 d-major
W_PC = N * 3 // 2
OFF_HL = OFF_PC + B * W_PC             # per batch: [2, N] f16 (-hi, -lo)
W_HL = N
OFF_BIAS = OFF_HL + B * W_HL           # [B*NT, 128] f32
W_BIAS = B * NT * 128
BLOB_LEN = OFF_BIAS + W_BIAS

_CACHE = {}


def _hi_lanes(key):
    """[128, 8192] bf16 view of the high u16 lane of each f32 key word."""
    return key.bitcast(BF16).rearrange("p (n two) -> p n two", two=2)[:, :, 1]


def _lo_lanes(key):
    """[128, 8192] u16 view of the low (index) lane of each f32 key word."""
    return key.bitcast(U16).rearrange("p (n two) -> p n two", two=2)[:, :, 0]


def _compute_tile(nc, t, b, spool, ppool, key, Cp16, fio3, lhsT, nbias):
    """Matmul + packed-key top-8 + radius fixup for per-batch tile t."""
    dynamic = not isinstance(t, int)
    gl = t + b * NT
    tsl = ds(gl, 1) if dynamic else slice(gl, gl + 1)
    hi = _hi_lanes(key)
    for cp in range(NCPY):
        ps = ppool.tile([128, CPY], F32, tag="ps")
        for k in range(CPY // CH):
            ch = cp * (CPY // CH) + k
            nc.tensor.matmul(out=ps[:, ts(k, CH)], lhsT=lhsT,
                             rhs=Cp16[:, b * N + ch * CH : b * N + (ch + 1) * CH],
                             start=True, stop=True)
        # bf16 quality level into the high lanes: q = SCALE*s + (BIAS0 - S|q|^2)
        nc.scalar.activation(out=hi[:, ts(cp, CPY)], in_=ps[:, :],
                             func=mybir.ActivationFunctionType.Identity,
                             bias=nbias, scale=float(SCALE))

    tv = spool.tile([128, 8], F32)
    nc.vector.max(out=tv[:, :], in_=key[:, :])
    # u = key*8192 (exact int), clamped >= 0; low u16 = global mask row
    uc = spool.tile([128, 8], U32)
    nc.vector.tensor_scalar(out=uc[:, :], in0=tv[:, :], scalar1=8192.0,
                            scalar2=0.0, op0=mybir.AluOpType.mult,
                            op1=mybir.AluOpType.max)
    raw = spool.tile([128, 8], U32)
    nc.vector.tensor_copy(
        raw[:, :],
        uc.bitcast(U16).rearrange("p (e two) -> p e two", two=2)[:, :, 0])
    kp = spool.tile([128, 8], U32)
    nc.vector.tensor_scalar(out=kp[:, :], in0=tv[:, :],
                            scalar1=THR + float(b), scalar2=None,
                            op0=mybir.AluOpType.is_ge)
    nk = spool.tile([128, 8], U32)
    nc.vector.tensor_scalar(out=nk[:, :], in0=tv[:, :],
                            scalar1=THR + float(b), scalar2=None,
                            op0=mybir.AluOpType.is_lt)
    # fio = kp*raw + (1-kp)*self   (all terms non-negative)
    selfbc = raw[:, 0:1].to_broadcast([128, 8])
    t1 = spool.tile([128, 8], U32)
    nc.vector.tensor_tensor(out=t1[:, :], in0=kp[:, :], in1=raw[:, :],
                            op=mybir.AluOpType.mult)
    t2 = spool.tile([128, 8], U32)
    nc.vector.tensor_tensor(out=t2[:, :], in0=nk[:, :], in1=selfbc,
                            op=mybir.AluOpType.mult)
    fio = fio3[:, tsl, :].squeeze(1)
    nc.vector.tensor_tensor(out=fio, in0=t1[:, :], in1=t2[:, :],
                            op=mybir.AluOpType.add)


def _gather_issue(nc, spool, mask_g, fio_phys, u):
    """Issue the 7 neighbor-row gathers for one tile; returns the gt tile."""
    gt = spool.tile([128, KNN - 1, KS], BF16, tag=f"gt{u}")
    for j in range(1, KNN):
        nc.gpsimd.indirect_dma_start(
            out=gt[:, j - 1, :], out_offset=None, in_=mask_g,
            in_offset=IndirectOffsetOnAxis(ap=fio_phys[:, j : j + 1], axis=0),
        )
    return gt


def _gather_consume(nc, t, b, spool, mql3, gt, lt_all, u):
    """L1-diff of one tile's gathered rows into its lt_all slot."""
    dynamic = not isinstance(t, int)
    gl = t + b * NT
    tsl = ds(gl, 1) if dynamic else slice(gl, gl + 1)
    mq_bc = mql3[:, tsl, :].to_broadcast([128, KNN - 1, KS])
    df = spool.tile([128, KNN - 1, KS], F32, tag=f"df{u}")
    nc.gpsimd.tensor_tensor(out=df[:, :, :], in0=gt[:, :, :], in1=mq_bc,
                            op=mybir.AluOpType.subtract)
    nc.vector.tensor_reduce(out=lt_all[:, tsl], in_=df[:, :, :],
                            axis=mybir.AxisListType.XY,
                            op=mybir.AluOpType.add,
                            apply_absolute_value=True)


def _body(tc, blob, loss_out, unroll=UNROLL):
    nc = tc.nc
    bap = blob.ap()
    mask_g = bap[OFF_MASK : OFF_MASK + W_MASK].bitcast(BF16).rearrange(
        "(n s) -> n s", s=KS)
    pc_all = [
        bap[OFF_PC + b * W_PC : OFF_PC + (b + 1) * W_PC]
        .bitcast(F16).rearrange("(d n) -> d n", d=3)
        for b in range(B)
    ]
    hl_all = [
        bap[OFF_HL + b * W_HL : OFF_HL + (b + 1) * W_HL]
        .bitcast(F16).rearrange("(d n) -> d n", d=2)
        for b in range(B)
    ]
    mask_qT = [
        bap[b * N * KS // 2 : (b + 1) * N * KS // 2].bitcast(BF16).rearrange(
            "(t p s) -> p t s", p=128, s=KS)
        for b in range(B)
    ]
    bias_g = bap[OFF_BIAS : OFF_BIAS + W_BIAS].bitcast(F32).rearrange(
        "(t p) -> p t", p=128)

    import contextlib
    with contextlib.ExitStack() as ctx:
        cpool = ctx.enter_context(tc.tile_pool(name="const", bufs=1))
        spool = ctx.enter_context(tc.tile_pool(name="small", bufs=3))
        ppool = ctx.enter_context(tc.tile_pool(name="psum", bufs=2, space="PSUM"))

        keyA = cpool.tile([128, N], F32)
        keyB = cpool.tile([128, N], F32)
        Cp16 = cpool.tile([5, B * N], F16)
        Qs16 = cpool.tile([5, B * N], F16)
        mql = cpool.tile([128, B * NT * KS], BF16)
        fioall = cpool.tile([128, B * NT * 8], U32)
        nsbias = cpool.tile([128, B * NT], F32)
        lt_all = cpool.tile([128, B * NT], F32)
        acc = cpool.tile([128, 1], F32)
        nc.vector.memset(Qs16[0:5, :], 1.0)
        for key in (keyA, keyB):
            nc.gpsimd.iota(_lo_lanes(key), [[1, N]], base=0,
                           channel_multiplier=0)
        nc.sync.dma_start(out=nsbias[:, :], in_=bias_g)
        for b in range(B):
            nc.sync.dma_start(out=Cp16[0:3, b * N : (b + 1) * N], in_=pc_all[b])
            nc.sync.dma_start(out=Cp16[3:5, b * N : (b + 1) * N], in_=hl_all[b])
            nc.sync.dma_start(
                out=mql.rearrange("p (t s) -> p t s", s=KS)[:, b * NT : (b + 1) * NT, :],
                in_=mask_qT[b])
        nc.scalar.mul(Qs16[0:3, :], Cp16[0:3, :], 2.0)

        Qs3 = Qs16.rearrange("d (t p) -> d t p", p=128)   # [5, B*NT, 128]
        mql3 = mql.rearrange("p (t s) -> p t s", s=KS)
        fio3 = fioall.rearrange("p (t e) -> p t e", e=8)

        # Fully static software-pipelined emission: no For_i, no all-engine
        # barriers — per-tile dependencies flow through Tile semaphores only.
        # Gathers for tile g issue right after its fio is final; the L1
        # consume lags LAG tiles so the gather DMAs are long done.
        LAG = 3
        keys = (keyA, keyB)
        pend = []   # (global_tile, b, gt, slot)
        gslot = 0
        for b in range(B):
            if b == 1:
                for key in (keyA, keyB):
                    nc.vector.tensor_scalar(out=_lo_lanes(key),
                                            in0=_lo_lanes(key),
                                            scalar1=N, scalar2=None,
                                            op0=mybir.AluOpType.add)
            for t in range(NT):
                _compute_tile(nc, t, b, spool, ppool, keys[t % 2], Cp16,
                              fio3,
                              lhsT=Qs3[:, b * NT + t : b * NT + t + 1, :].squeeze(1),
                              nbias=nsbias[:, b * NT + t : b * NT + t + 1])
                gt = _gather_issue(
                    nc, spool, mask_g,
                    fio3[:, t + b * NT : t + b * NT + 1, :].squeeze(1),
                    gslot % (LAG + 1))
                pend.append((t, b, gt))
                gslot += 1
                if len(pend) > LAG:
                    pt, pb, pgt = pend.pop(0)
                    _gather_consume(nc, pt, pb, spool, mql3, pgt, lt_all,
                                    gslot % (LAG + 1))
        for pt, pb, pgt in pend:
            gslot += 1
            _gather_consume(nc, pt, pb, spool, mql3, pgt, lt_all,
                            gslot % (LAG + 1))

        nc.vector.tensor_reduce(out=acc[:, :], in_=lt_all[:, :],
                                axis=mybir.AxisListType.X,
                                op=mybir.AluOpType.add)
        nc.sync.dma_start(out=loss_out.ap()[:, :], in_=acc[:, :])


def build_nc(unroll=UNROLL):
    nc = bacc.Bacc("TRN2", target_bir_lowering=False, debug=False,
                   num_devices=NCORES, enable_partition_id=False)
    blob = nc.dram_tensor("blob", [BLOB_LEN], U32, kind="ExternalInput")
    loss_out = nc.dram_tensor("loss_out", [128, 1], F32, kind="ExternalOutput")
    with tile.TileContext(nc) as tc:
        _body(tc, blob, loss_out, unroll=unroll)
    nc.compile()
    return nc


def make_in_maps(pc, mask):
    import ml_dtypes
    pc16 = np.asarray(np.asarray(pc), np.float32).astype(np.float16)
    maskb = np.asarray(np.asarray(mask), np.float32).astype(ml_dtypes.bfloat16)
    parts = [maskb.reshape(-1).view(np.uint32)]
    for b in range(B):
        parts.append(np.ascontiguousarray(pc16[b].T).reshape(-1).view(np.uint32))
    hlparts, biasparts = [], []
    for b in range(B):
        c32 = pc16[b].astype(np.float32)
        csq = (c32 * c32).sum(-1)                       # f32 |c|^2 of fp16 coords
        hi = csq.astype(np.float16)
        lo = (csq - hi.astype(np.float32)).astype(np.float16)
        hlparts.append(np.concatenate([-hi, -lo]).reshape(-1).view(np.uint32))
        # bias rows in [t, p] order: query index = t*128 + p
        bias = (np.float32(BIAS0) - np.float32(SCALE) * csq).astype(np.float32)
        biasparts.append(bias.reshape(NT, 128))
    parts.extend(hlparts)
    parts.append(np.concatenate(biasparts, axis=0).reshape(-1).view(np.uint32))
    return [{"blob": np.concatenate(parts)}]


def kernel(pc, mask):
    if "nc" not in _CACHE:
        _CACHE["nc"] = build_nc()
    nc = _CACHE["nc"]
    res = run_bass_kernel_spmd(nc, make_in_maps(pc, mask), list(range(NCORES)))
    total = 0.0
    for r in res.results:
        total += r["loss_out"].astype(np.float64).sum()
    return np.float32(total / (B * N * KNN))


# revision 3
# speedup vs baseline: 1.1666x; 1.1666x over previous
"""KnnLoss Trainium2 kernel (v15) — single-core, packed-key top-8, fully unrolled.

Math per batch: scores s = 2 q.c - |c|^2 via fp16 matmul (hi/lo split of
|c|^2, host-precomputed). The PSUM->SBUF activation writes bf16 "quality
levels" q = 2040 - S*d2 (scale=S, per-query bias) into the HIGH u16 lanes
of a persistent f32 key tile whose LOW u16 lanes hold the candidate index
(+8192 for batch 1), so each f32 word reads as key = q_bf16 + idx*2^-13.
ONE DVE Max pass then yields top-8 values with indices embedded
(u = key*8192 is an exact integer whose low 16 bits are the global mask
row). Out-of-radius slots (key < THR) are replaced by the self index
arithmetically (no CopyPredicated). The 7 neighbor mask rows are fetched
in ONE merged indirect DMA per tile; L1-diff accumulated on device.

Single core processes both batches (multi-device dispatch through the
axon tunnel costs ~3x a single-device dispatch); the key-tile index lanes
are bumped by +8192 between batches so gathers hit the [2N, KS] table.
"""

import numpy as np

import concourse.bass as bass
import concourse.mybir as mybir
import concourse.tile as tile
from concourse import bacc
from concourse.bass import IndirectOffsetOnAxis, ds, ts
from concourse.bass_utils import run_bass_kernel_spmd

B = 2
N = 8192
KS = 16
KNN = 8

NCORES = 1
UNROLL = 8
NT = N // 128          # 64 tiles per batch
NB = B * N             # mask table rows

F32 = mybir.dt.float32
F16 = mybir.dt.float16
BF16 = mybir.dt.bfloat16
U32 = mybir.dt.uint32
U16 = mybir.dt.uint16

CH = 512
CPY = 2048
NCPY = N // CPY

SCALE = np.float32(79921.875)          # key units per unit d2
BIAS0 = np.float32(2040.0)             # self (d2=0) key level
R2 = np.float32(0.1) * np.float32(0.1)
THR = float(BIAS0 - SCALE * R2)        # in-radius iff key >= THR (+b for batch b)

# blob layout in u32 words
OFF_MASK = 0
W_MASK = NB * KS // 2
OFF_PC = W_MASK                        # per batch: [3, N] f16 d-major
W_PC = N * 3 // 2
OFF_HL = OFF_PC + B * W_PC             # per batch: [2, N] f16 (-hi, -lo)
W_HL = N
OFF_BIAS = OFF_HL + B * W_HL           # [B*NT, 128] f32
W_BIAS = B * NT * 128
BLOB_LEN = OFF_BIAS + W_BIAS

_CACHE = {}


def _hi_lanes(key):
    """[128, 8192] bf16 view of the high u16 lane of each f32 key word."""
    return key.bitcast(BF16).rearrange("p (n two) -> p n two", two=2)[:, :, 1]


def _lo_lanes(key):
    """[128, 8192] u16 view of the low (index) lane of each f32 key word."""
    return key.bitcast(U16).rearrange("p (n two) -> p n two", two=2)[:, :, 0]


def _compute_tile(nc, t, b, spool, ppool, key, Cp16, fio3, lhsT, nbias):
    """Matmul + packed-key top-8 + radius fixup for per-batch tile t."""
    dynamic = not isinstance(t, int)
    gl = t + b * NT
    tsl = ds(gl, 1) if dynamic else slice(gl, gl + 1)
    hi = _hi_lanes(key)
    for cp in range(NCPY):
        ps = ppool.tile([128, CPY], F32, tag="ps")
        for k in range(CPY // CH):
            ch = cp * (CPY // CH) + k
            nc.tensor.matmul(out=ps[:, ts(k, CH)], lhsT=lhsT,
                             rhs=Cp16[:, b * N + ch * CH : b * N + (ch + 1) * CH],
                             start=True, stop=True)
        # bf16 quality level into the high lanes: q = SCALE*s + (BIAS0 - S|q|^2)
        nc.scalar.activation(out=hi[:, ts(cp, CPY)], in_=ps[:, :],
                             func=mybir.ActivationFunctionType.Identity,
                             bias=nbias, scale=float(SCALE))

    tv = spool.tile([128, 8], F32)
    nc.vector.max(out=tv[:, :], in_=key[:, :])
    # u = key*8192 (exact int), clamped >= 0; low u16 = global mask row
    uc = spool.tile([128, 8], U32)
    nc.vector.tensor_scalar(out=uc[:, :], in0=tv[:, :], scalar1=8192.0,
                            scalar2=0.0, op0=mybir.AluOpType.mult,
                            op1=mybir.AluOpType.max)
    # out-of-radius slots -> self: self's packed key is the row max, so
    # fio_u = max(u, (key < THR) * self_u) selects self exactly there.
    nk = spool.tile([128, 8], U32)
    nc.vector.tensor_scalar(out=nk[:, :], in0=tv[:, :],
                            scalar1=THR + float(b), scalar2=None,
                            op0=mybir.AluOpType.is_lt)
    selfbc = uc[:, 0:1].to_broadcast([128, 8])
    t1 = spool.tile([128, 8], U32)
    nc.vector.tensor_tensor(out=t1[:, :], in0=nk[:, :], in1=selfbc,
                            op=mybir.AluOpType.mult)
    fu = spool.tile([128, 8], U32)
    nc.vector.tensor_tensor(out=fu[:, :], in0=uc[:, :], in1=t1[:, :],
                            op=mybir.AluOpType.max)
    fio = fio3[:, tsl, :].squeeze(1)
    nc.vector.tensor_copy(
        fio, fu.bitcast(U16).rearrange("p (e two) -> p e two", two=2)[:, :, 0])


def _gather_issue(nc, spool, mask_g, fio_phys, u):
    """Issue the 7 neighbor-row gathers for one tile; returns the gt tile."""
    gt = spool.tile([128, KNN - 1, KS], BF16, tag=f"gt{u}")
    for j in range(1, KNN):
        nc.gpsimd.indirect_dma_start(
            out=gt[:, j - 1, :], out_offset=None, in_=mask_g,
            in_offset=IndirectOffsetOnAxis(ap=fio_phys[:, j : j + 1], axis=0),
        )
    return gt


def _gather_consume(nc, t, b, spool, mql3, gt, lt_all, u):
    """L1-diff of one tile's gathered rows into its lt_all slot."""
    dynamic = not isinstance(t, int)
    gl = t + b * NT
    tsl = ds(gl, 1) if dynamic else slice(gl, gl + 1)
    mq_bc = mql3[:, tsl, :].to_broadcast([128, KNN - 1, KS])
    df = spool.tile([128, KNN - 1, KS], F32, tag=f"df{u}")
    nc.gpsimd.tensor_tensor(out=df[:, :, :], in0=gt[:, :, :], in1=mq_bc,
                            op=mybir.AluOpType.subtract)
    nc.vector.tensor_reduce(out=lt_all[:, tsl], in_=df[:, :, :],
                            axis=mybir.AxisListType.XY,
                            op=mybir.AluOpType.add,
                            apply_absolute_value=True)


def _body(tc, blob, loss_out, unroll=UNROLL):
    nc = tc.nc
    bap = blob.ap()
    mask_g = bap[OFF_MASK : OFF_MASK + W_MASK].bitcast(BF16).rearrange(
        "(n s) -> n s", s=KS)
    pc_all = [
        bap[OFF_PC + b * W_PC : OFF_PC + (b + 1) * W_PC]
        .bitcast(F16).rearrange("(d n) -> d n", d=3)
        for b in range(B)
    ]
    hl_all = [
        bap[OFF_HL + b * W_HL : OFF_HL + (b + 1) * W_HL]
        .bitcast(F16).rearrange("(d n) -> d n", d=2)
        for b in range(B)
    ]
    mask_qT = [
        bap[b * N * KS // 2 : (b + 1) * N * KS // 2].bitcast(BF16).rearrange(
            "(t p s) -> p t s", p=128, s=KS)
        for b in range(B)
    ]
    bias_g = bap[OFF_BIAS : OFF_BIAS + W_BIAS].bitcast(F32).rearrange(
        "(t p) -> p t", p=128)

    import contextlib
    with contextlib.ExitStack() as ctx:
        cpool = ctx.enter_context(tc.tile_pool(name="const", bufs=1))
        spool = ctx.enter_context(tc.tile_pool(name="small", bufs=3))
        ppool = ctx.enter_context(tc.tile_pool(name="psum", bufs=2, space="PSUM"))

        keyA = cpool.tile([128, N], F32)
        keyB = cpool.tile([128, N], F32)
        Cp16 = cpool.tile([5, B * N], F16)
        Qs16 = cpool.tile([5, B * N], F16)
        mql = cpool.tile([128, B * NT * KS], BF16)
        fioall = cpool.tile([128, B * NT * 8], U32)
        nsbias = cpool.tile([128, B * NT], F32)
        lt_all = cpool.tile([128, B * NT], F32)
        acc = cpool.tile([128, 1], F32)
        nc.vector.memset(Qs16[0:5, :], 1.0)
        for key in (keyA, keyB):
            nc.gpsimd.iota(_lo_lanes(key), [[1, N]], base=0,
                           channel_multiplier=0)
        nc.sync.dma_start(out=nsbias[:, :], in_=bias_g)
        for b in range(B):
            nc.sync.dma_start(out=Cp16[0:3, b * N : (b + 1) * N], in_=pc_all[b])
            nc.sync.dma_start(out=Cp16[3:5, b * N : (b + 1) * N], in_=hl_all[b])
            nc.sync.dma_start(
                out=mql.rearrange("p (t s) -> p t s", s=KS)[:, b * NT : (b + 1) * NT, :],
                in_=mask_qT[b])
        nc.scalar.mul(Qs16[0:3, :], Cp16[0:3, :], 2.0)

        Qs3 = Qs16.rearrange("d (t p) -> d t p", p=128)   # [5, B*NT, 128]
        mql3 = mql.rearrange("p (t s) -> p t s", s=KS)
        fio3 = fioall.rearrange("p (t e) -> p t e", e=8)

        # Fully static software-pipelined emission: no For_i, no all-engine
        # barriers — per-tile dependencies flow through Tile semaphores only.
        # Gathers for tile g issue right after its fio is final; the L1
        # consume lags LAG tiles so the gather DMAs are long done.
        LAG = 3
        keys = (keyA, keyB)
        pend = []   # (global_tile, b, gt, slot)
        gslot = 0
        for b in range(B):
            if b == 1:
                for key in (keyA, keyB):
                    nc.vector.tensor_scalar(out=_lo_lanes(key),
                                            in0=_lo_lanes(key),
                                            scalar1=N, scalar2=None,
                                            op0=mybir.AluOpType.add)
            for t in range(NT):
                _compute_tile(nc, t, b, spool, ppool, keys[t % 2], Cp16,
                              fio3,
                              lhsT=Qs3[:, b * NT + t : b * NT + t + 1, :].squeeze(1),
                              nbias=nsbias[:, b * NT + t : b * NT + t + 1])
                gt = _gather_issue(
                    nc, spool, mask_g,
                    fio3[:, t + b * NT : t + b * NT + 1, :].squeeze(1),
                    gslot % (LAG + 1))
                pend.append((t, b, gt))
                gslot += 1
                if len(pend) > LAG:
                    pt, pb, pgt = pend.pop(0)
                    _gather_consume(nc, pt, pb, spool, mql3, pgt, lt_all,
                                    gslot % (LAG + 1))
        for pt, pb, pgt in pend:
            gslot += 1
            _gather_consume(nc, pt, pb, spool, mql3, pgt, lt_all,
                            gslot % (LAG + 1))

        nc.vector.tensor_reduce(out=acc[:, :], in_=lt_all[:, :],
                                axis=mybir.AxisListType.X,
                                op=mybir.AluOpType.add)
        nc.sync.dma_start(out=loss_out.ap()[:, :], in_=acc[:, :])


def build_nc(unroll=UNROLL):
    nc = bacc.Bacc("TRN2", target_bir_lowering=False, debug=False,
                   num_devices=NCORES, enable_partition_id=False)
    blob = nc.dram_tensor("blob", [BLOB_LEN], U32, kind="ExternalInput")
    loss_out = nc.dram_tensor("loss_out", [128, 1], F32, kind="ExternalOutput")
    with tile.TileContext(nc) as tc:
        _body(tc, blob, loss_out, unroll=unroll)
    nc.compile()
    return nc


def make_in_maps(pc, mask):
    import ml_dtypes
    pc16 = np.asarray(np.asarray(pc), np.float32).astype(np.float16)
    maskb = np.asarray(np.asarray(mask), np.float32).astype(ml_dtypes.bfloat16)
    parts = [maskb.reshape(-1).view(np.uint32)]
    for b in range(B):
        parts.append(np.ascontiguousarray(pc16[b].T).reshape(-1).view(np.uint32))
    hlparts, biasparts = [], []
    for b in range(B):
        c32 = pc16[b].astype(np.float32)
        csq = (c32 * c32).sum(-1)                       # f32 |c|^2 of fp16 coords
        hi = csq.astype(np.float16)
        lo = (csq - hi.astype(np.float32)).astype(np.float16)
        hlparts.append(np.concatenate([-hi, -lo]).reshape(-1).view(np.uint32))
        # bias rows in [t, p] order: query index = t*128 + p
        bias = (np.float32(BIAS0) - np.float32(SCALE) * csq).astype(np.float32)
        biasparts.append(bias.reshape(NT, 128))
    parts.extend(hlparts)
    parts.append(np.concatenate(biasparts, axis=0).reshape(-1).view(np.uint32))
    return [{"blob": np.concatenate(parts)}]


def kernel(pc, mask):
    if "nc" not in _CACHE:
        _CACHE["nc"] = build_nc()
    nc = _CACHE["nc"]
    res = run_bass_kernel_spmd(nc, make_in_maps(pc, mask), list(range(NCORES)))
    total = 0.0
    for r in res.results:
        total += r["loss_out"].astype(np.float64).sum()
    return np.float32(total / (B * N * KNN))


# revision 5
# speedup vs baseline: 2.2646x; 1.9411x over previous
"""KnnLoss Trainium2 kernel (v15) — single-core, packed-key top-8, fully unrolled.

Math per batch: scores s = 2 q.c - |c|^2 via fp16 matmul (hi/lo split of
|c|^2, host-precomputed). The PSUM->SBUF activation writes bf16 "quality
levels" q = 2040 - S*d2 (scale=S, per-query bias) into the HIGH u16 lanes
of a persistent f32 key tile whose LOW u16 lanes hold the candidate index
(+8192 for batch 1), so each f32 word reads as key = q_bf16 + idx*2^-13.
ONE DVE Max pass then yields top-8 values with indices embedded
(u = key*8192 is an exact integer whose low 16 bits are the global mask
row). Out-of-radius slots (key < THR) are replaced by the self index
arithmetically (no CopyPredicated). The 7 neighbor mask rows are fetched
in ONE merged indirect DMA per tile; L1-diff accumulated on device.

Single core processes both batches (multi-device dispatch through the
axon tunnel costs ~3x a single-device dispatch); the key-tile index lanes
are bumped by +8192 between batches so gathers hit the [2N, KS] table.
"""

import numpy as np

import concourse.bass as bass
import concourse.mybir as mybir
import concourse.tile as tile
from concourse import bacc
from concourse.bass import IndirectOffsetOnAxis, ds, ts
from concourse.bass_utils import run_bass_kernel_spmd

B = 2
N = 8192
KS = 16
KNN = 8

NCORES = 1
UNROLL = 8
NT = N // 128          # 64 tiles per batch
NB = B * N             # mask table rows

F32 = mybir.dt.float32
F16 = mybir.dt.float16
BF16 = mybir.dt.bfloat16
U32 = mybir.dt.uint32
U16 = mybir.dt.uint16

CH = 512
CPY = 2048
NCPY = N // CPY

SCALE = np.float32(79921.875)          # key units per unit d2
BIAS0 = np.float32(2040.0)             # self (d2=0) key level
R2 = np.float32(0.1) * np.float32(0.1)
THR = float(BIAS0 - SCALE * R2)        # in-radius iff key >= THR (+b for batch b)

# blob layout in u32 words
OFF_MASK = 0
W_MASK = NB * KS // 2
OFF_PC = W_MASK                        # per batch: [3, N] f16 d-major
W_PC = N * 3 // 2
OFF_HL = OFF_PC + B * W_PC             # per batch: [2, N] f16 (-hi, -lo)
W_HL = N
OFF_BIAS = OFF_HL + B * W_HL           # [B*NT, 128] f32
W_BIAS = B * NT * 128
BLOB_LEN = OFF_BIAS + W_BIAS

_CACHE = {}


def _hi_lanes(key):
    """[128, 8192] bf16 view of the high u16 lane of each f32 key word."""
    return key.bitcast(BF16).rearrange("p (n two) -> p n two", two=2)[:, :, 1]


def _lo_lanes(key):
    """[128, 8192] u16 view of the low (index) lane of each f32 key word."""
    return key.bitcast(U16).rearrange("p (n two) -> p n two", two=2)[:, :, 0]


def _compute_tile(nc, t, b, spool, ppool, key, Cp16, fio3, lhsT, nbias):
    """Matmul + packed-key top-8 + radius fixup for per-batch tile t."""
    dynamic = not isinstance(t, int)
    gl = t + b * NT
    tsl = ds(gl, 1) if dynamic else slice(gl, gl + 1)
    hi = _hi_lanes(key)
    for cp in range(NCPY):
        ps = ppool.tile([128, CPY], F32, tag="ps")
        for k in range(CPY // CH):
            ch = cp * (CPY // CH) + k
            nc.tensor.matmul(out=ps[:, ts(k, CH)], lhsT=lhsT,
                             rhs=Cp16[:, b * N + ch * CH : b * N + (ch + 1) * CH],
                             start=True, stop=True)
        # bf16 quality level into the high lanes: q = SCALE*s + (BIAS0 - S|q|^2)
        nc.scalar.activation(out=hi[:, ts(cp, CPY)], in_=ps[:, :],
                             func=mybir.ActivationFunctionType.Identity,
                             bias=nbias, scale=float(SCALE))

    tv = spool.tile([128, 8], F32)
    nc.vector.max(out=tv[:, :], in_=key[:, :])
    # u = key*8192 (exact int), clamped >= 0; low u16 = global mask row
    uc = spool.tile([128, 8], U32)
    nc.vector.tensor_scalar(out=uc[:, :], in0=tv[:, :], scalar1=8192.0,
                            scalar2=0.0, op0=mybir.AluOpType.mult,
                            op1=mybir.AluOpType.max)
    # out-of-radius slots -> self: self's packed key is the row max, so
    # fio_u = max(u, (key < THR) * self_u) selects self exactly there.
    nk = spool.tile([128, 8], U32)
    nc.vector.tensor_scalar(out=nk[:, :], in0=tv[:, :],
                            scalar1=THR + float(b), scalar2=None,
                            op0=mybir.AluOpType.is_lt)
    selfbc = uc[:, 0:1].to_broadcast([128, 8])
    t1 = spool.tile([128, 8], U32)
    nc.vector.tensor_tensor(out=t1[:, :], in0=nk[:, :], in1=selfbc,
                            op=mybir.AluOpType.mult)
    fu = spool.tile([128, 8], U32)
    nc.vector.tensor_tensor(out=fu[:, :], in0=uc[:, :], in1=t1[:, :],
                            op=mybir.AluOpType.max)
    fio = fio3[:, tsl, :].squeeze(1)
    nc.vector.tensor_copy(
        fio, fu.bitcast(U16).rearrange("p (e two) -> p e two", two=2)[:, :, 0])


def _gather_issue(nc, spool, mask_g, fio_phys, u):
    """Issue the 7 neighbor-row gathers for one tile; returns the gt tile."""
    gt = spool.tile([128, KNN - 1, KS], BF16, tag=f"gt{u}")
    for j in range(1, KNN):
        nc.gpsimd.indirect_dma_start(
            out=gt[:, j - 1, :], out_offset=None, in_=mask_g,
            in_offset=IndirectOffsetOnAxis(ap=fio_phys[:, j : j + 1], axis=0),
        )
    return gt


def _gather_consume(nc, t, b, spool, mql3, gt, lt_all, u):
    """L1-diff of one tile's gathered rows into its lt_all slot."""
    dynamic = not isinstance(t, int)
    gl = t + b * NT
    tsl = ds(gl, 1) if dynamic else slice(gl, gl + 1)
    mq_bc = mql3[:, tsl, :].to_broadcast([128, KNN - 1, KS])
    df = spool.tile([128, KNN - 1, KS], F32, tag=f"df{u}")
    nc.gpsimd.tensor_tensor(out=df[:, :, :], in0=gt[:, :, :], in1=mq_bc,
                            op=mybir.AluOpType.subtract)
    nc.vector.tensor_reduce(out=lt_all[:, tsl], in_=df[:, :, :],
                            axis=mybir.AxisListType.XY,
                            op=mybir.AluOpType.add,
                            apply_absolute_value=True)


def _body(tc, blob, loss_out, unroll=UNROLL):
    nc = tc.nc
    bap = blob.ap()
    mask_g = bap[OFF_MASK : OFF_MASK + W_MASK].bitcast(BF16).rearrange(
        "(n s) -> n s", s=KS)
    pc_all = [
        bap[OFF_PC + b * W_PC : OFF_PC + (b + 1) * W_PC]
        .bitcast(F16).rearrange("(d n) -> d n", d=3)
        for b in range(B)
    ]
    hl_all = [
        bap[OFF_HL + b * W_HL : OFF_HL + (b + 1) * W_HL]
        .bitcast(F16).rearrange("(d n) -> d n", d=2)
        for b in range(B)
    ]
    mask_qT = [
        bap[b * N * KS // 2 : (b + 1) * N * KS // 2].bitcast(BF16).rearrange(
            "(t p s) -> p t s", p=128, s=KS)
        for b in range(B)
    ]
    bias_g = bap[OFF_BIAS : OFF_BIAS + W_BIAS].bitcast(F32).rearrange(
        "(t p) -> p t", p=128)

    import contextlib
    with contextlib.ExitStack() as ctx:
        cpool = ctx.enter_context(tc.tile_pool(name="const", bufs=1))
        spool = ctx.enter_context(tc.tile_pool(name="small", bufs=3))
        ppool = ctx.enter_context(tc.tile_pool(name="psum", bufs=2, space="PSUM"))

        keyA = cpool.tile([128, N], F32)
        keyB = cpool.tile([128, N], F32)
        Cp16 = cpool.tile([5, B * N], F16)
        Qs16 = cpool.tile([5, B * N], F16)
        mql = cpool.tile([128, B * NT * KS], BF16)
        fioall = cpool.tile([128, B * NT * 8], U32)
        nsbias = cpool.tile([128, B * NT], F32)
        lt_all = cpool.tile([128, B * NT], F32)
        acc = cpool.tile([128, 1], F32)
        nc.vector.memset(Qs16[0:5, :], 1.0)
        for key in (keyA, keyB):
            nc.gpsimd.iota(_lo_lanes(key), [[1, N]], base=0,
                           channel_multiplier=0)
        nc.sync.dma_start(out=nsbias[:, :], in_=bias_g)
        for b in range(B):
            nc.sync.dma_start(out=Cp16[0:3, b * N : (b + 1) * N], in_=pc_all[b])
            nc.sync.dma_start(out=Cp16[3:5, b * N : (b + 1) * N], in_=hl_all[b])
            nc.sync.dma_start(
                out=mql.rearrange("p (t s) -> p t s", s=KS)[:, b * NT : (b + 1) * NT, :],
                in_=mask_qT[b])
        nc.scalar.mul(Qs16[0:3, :], Cp16[0:3, :], 2.0)

        Qs3 = Qs16.rearrange("d (t p) -> d t p", p=128)   # [5, B*NT, 128]
        mql3 = mql.rearrange("p (t s) -> p t s", s=KS)
        fio3 = fioall.rearrange("p (t e) -> p t e", e=8)

        # Fully static software-pipelined emission: no For_i, no all-engine
        # barriers — per-tile dependencies flow through Tile semaphores only.
        # Gathers for tile g issue right after its fio is final; the L1
        # consume lags LAG tiles so the gather DMAs are long done.
        LAG = 3
        keys = (keyA, keyB)
        pend = []   # (global_tile, b, gt, slot)
        gslot = 0
        for b in range(B):
            if b == 1:
                for key in (keyA, keyB):
                    nc.vector.tensor_scalar(out=_lo_lanes(key),
                                            in0=_lo_lanes(key),
                                            scalar1=N, scalar2=None,
                                            op0=mybir.AluOpType.add)
            for t in range(NT):
                _compute_tile(nc, t, b, spool, ppool, keys[t % 2], Cp16,
                              fio3,
                              lhsT=Qs3[:, b * NT + t : b * NT + t + 1, :].squeeze(1),
                              nbias=nsbias[:, b * NT + t : b * NT + t + 1])
                gt = _gather_issue(
                    nc, spool, mask_g,
                    fio3[:, t + b * NT : t + b * NT + 1, :].squeeze(1),
                    gslot % (LAG + 1))
                pend.append((t, b, gt))
                gslot += 1
                if len(pend) > LAG:
                    pt, pb, pgt = pend.pop(0)
                    _gather_consume(nc, pt, pb, spool, mql3, pgt, lt_all,
                                    gslot % (LAG + 1))
        for pt, pb, pgt in pend:
            gslot += 1
            _gather_consume(nc, pt, pb, spool, mql3, pgt, lt_all,
                            gslot % (LAG + 1))

        nc.vector.tensor_reduce(out=acc[:, :], in_=lt_all[:, :],
                                axis=mybir.AxisListType.X,
                                op=mybir.AluOpType.add)
        nc.sync.dma_start(out=loss_out.ap()[:, :], in_=acc[:, :])


def build_nc(unroll=UNROLL):
    nc = bacc.Bacc("TRN2", target_bir_lowering=False, debug=False,
                   num_devices=NCORES, enable_partition_id=False)
    blob = nc.dram_tensor("blob", [BLOB_LEN], U32, kind="ExternalInput")
    loss_out = nc.dram_tensor("loss_out", [128, 1], F32, kind="ExternalOutput")
    with tile.TileContext(nc) as tc:
        _body(tc, blob, loss_out, unroll=unroll)
    nc.compile()
    return nc


def make_in_maps(pc, mask):
    import ml_dtypes
    pc16 = np.asarray(np.asarray(pc), np.float32).astype(np.float16)
    maskb = np.asarray(np.asarray(mask), np.float32).astype(ml_dtypes.bfloat16)
    parts = [maskb.reshape(-1).view(np.uint32)]
    for b in range(B):
        parts.append(np.ascontiguousarray(pc16[b].T).reshape(-1).view(np.uint32))
    hlparts, biasparts = [], []
    for b in range(B):
        c32 = pc16[b].astype(np.float32)
        csq = (c32 * c32).sum(-1)                       # f32 |c|^2 of fp16 coords
        hi = csq.astype(np.float16)
        lo = (csq - hi.astype(np.float32)).astype(np.float16)
        hlparts.append(np.concatenate([-hi, -lo]).reshape(-1).view(np.uint32))
        # bias rows in [t, p] order: query index = t*128 + p
        bias = (np.float32(BIAS0) - np.float32(SCALE) * csq).astype(np.float32)
        biasparts.append(bias.reshape(NT, 128))
    parts.extend(hlparts)
    parts.append(np.concatenate(biasparts, axis=0).reshape(-1).view(np.uint32))
    return [{"blob": np.concatenate(parts)}]


def kernel(pc, mask):
    if "nc" not in _CACHE:
        _CACHE["nc"] = build_nc()
    nc = _CACHE["nc"]
    res = run_bass_kernel_spmd(nc, make_in_maps(pc, mask), list(range(NCORES)))
    total = 0.0
    for r in res.results:
        total += r["loss_out"].astype(np.float64).sum()
    return np.float32(total / (B * N * KNN))


# revision 6
# speedup vs baseline: 2.7732x; 1.2246x over previous
"""KnnLoss Trainium2 kernel (v17 z-window) — single-core, packed-key top-8, fully unrolled.

Math per batch: scores s = 2 q.c - |c|^2 via fp16 matmul (hi/lo split of
|c|^2, host-precomputed). The PSUM->SBUF activation writes bf16 "quality
levels" q = 2040 - S*d2 (scale=S, per-query bias) into the HIGH u16 lanes
of a persistent f32 key tile whose LOW u16 lanes hold the candidate index
(+8192 for batch 1), so each f32 word reads as key = q_bf16 + idx*2^-13.
ONE DVE Max pass then yields top-8 values with indices embedded
(u = key*8192 is an exact integer whose low 16 bits are the global mask
row). Out-of-radius slots (key < THR) are replaced by the self index
arithmetically (no CopyPredicated). The 7 neighbor mask rows are fetched
in ONE merged indirect DMA per tile; L1-diff accumulated on device.

Single core processes both batches (multi-device dispatch through the
axon tunnel costs ~3x a single-device dispatch); the key-tile index lanes
are bumped by +8192 between batches so gathers hit the [2N, KS] table.
"""

import numpy as np

import concourse.bass as bass
import concourse.mybir as mybir
import concourse.tile as tile
from concourse import bacc
from concourse.bass import IndirectOffsetOnAxis, ds, ts
from concourse.bass_utils import run_bass_kernel_spmd

B = 2
N = 8192
KS = 16
KNN = 8

NCORES = 1
UNROLL = 8
NT = N // 128          # 64 tiles per batch
NB = B * N             # mask table rows

F32 = mybir.dt.float32
F16 = mybir.dt.float16
BF16 = mybir.dt.bfloat16
U32 = mybir.dt.uint32
U16 = mybir.dt.uint16

CH = 512
W = 2048            # z-window candidates per tile (host-sorted)
NCH_W = W // CH

SCALE = np.float32(79921.875)          # key units per unit d2
BIAS0 = np.float32(2040.0)             # self (d2=0) key level
R2 = np.float32(0.1) * np.float32(0.1)
THR = float(BIAS0 - SCALE * R2)        # in-radius iff key >= THR (+b for batch b)

# blob layout in u32 words
OFF_MASK = 0
W_MASK = NB * KS // 2
OFF_PC = W_MASK                        # per batch: [3, N] f16 d-major
W_PC = N * 3 // 2
OFF_HL = OFF_PC + B * W_PC             # per batch: [2, N] f16 (-hi, -lo)
W_HL = N
OFF_BIAS = OFF_HL + B * W_HL           # [B*NT, 128] f32
W_BIAS = B * NT * 128
BLOB_LEN = OFF_BIAS + W_BIAS

_CACHE = {}


def _hi_lanes(key):
    """[128, W] bf16 view of the high u16 lane of each f32 key word."""
    return key.bitcast(BF16).rearrange("p (n two) -> p n two", two=2)[:, :, 1]


def _lo_lanes(key):
    """[128, W] u16 view of the low (index) lane of each f32 key word."""
    return key.bitcast(U16).rearrange("p (n two) -> p n two", two=2)[:, :, 0]


def _compute_tile(nc, t, b, spool, ppool, key, Cp16, fio3, lhsT, nbias, lo_t):
    """Windowed matmul + packed-key top-8 + radius fixup for tile t."""
    gl = t + b * NT
    tsl = slice(gl, gl + 1)
    hi = _hi_lanes(key)
    base = b * N + lo_t
    ps = ppool.tile([128, W], F32, tag="ps")
    for k in range(NCH_W):
        nc.tensor.matmul(out=ps[:, ts(k, CH)], lhsT=lhsT,
                         rhs=Cp16[:, base + k * CH : base + (k + 1) * CH],
                         start=True, stop=True)
    # bf16 quality level into the high lanes: q = SCALE*s + (BIAS0 - S|q|^2)
    nc.scalar.activation(out=hi[:, :], in_=ps[:, :],
                         func=mybir.ActivationFunctionType.Identity,
                         bias=nbias, scale=float(SCALE))

    tv = spool.tile([128, 8], F32)
    nc.vector.max(out=tv[:, :], in_=key[:, :])
    # u = key*8192 (exact int), clamped >= 0; low u16 = global mask row
    uc = spool.tile([128, 8], U32)
    nc.vector.tensor_scalar(out=uc[:, :], in0=tv[:, :], scalar1=8192.0,
                            scalar2=0.0, op0=mybir.AluOpType.mult,
                            op1=mybir.AluOpType.max)
    # out-of-radius slots -> self: self's packed key is the row max, so
    # fio_u = max(u, (key < THR) * self_u) selects self exactly there.
    nk = spool.tile([128, 8], U32)
    nc.vector.tensor_scalar(out=nk[:, :], in0=tv[:, :],
                            scalar1=THR + float(b), scalar2=None,
                            op0=mybir.AluOpType.is_lt)
    selfbc = uc[:, 0:1].to_broadcast([128, 8])
    t1 = spool.tile([128, 8], U32)
    nc.vector.tensor_tensor(out=t1[:, :], in0=nk[:, :], in1=selfbc,
                            op=mybir.AluOpType.mult)
    fu = spool.tile([128, 8], U32)
    nc.vector.tensor_tensor(out=fu[:, :], in0=uc[:, :], in1=t1[:, :],
                            op=mybir.AluOpType.max)
    fio = fio3[:, tsl, :].squeeze(1)
    nc.vector.tensor_scalar(
        out=fio,
        in0=fu.bitcast(U16).rearrange("p (e two) -> p e two", two=2)[:, :, 0],
        scalar1=float(b * N + lo_t), scalar2=None, op0=mybir.AluOpType.add)


def _gather_issue(nc, spool, mask_g, fio_phys, u):
    """Issue the 7 neighbor-row gathers for one tile; returns the gt tile."""
    gt = spool.tile([128, KNN - 1, KS], BF16, tag=f"gt{u}")
    for j in range(1, KNN):
        nc.gpsimd.indirect_dma_start(
            out=gt[:, j - 1, :], out_offset=None, in_=mask_g,
            in_offset=IndirectOffsetOnAxis(ap=fio_phys[:, j : j + 1], axis=0),
        )
    return gt


def _gather_consume(nc, t, b, spool, mql3, gt, lt_all, u):
    """L1-diff of one tile's gathered rows into its lt_all slot."""
    dynamic = not isinstance(t, int)
    gl = t + b * NT
    tsl = ds(gl, 1) if dynamic else slice(gl, gl + 1)
    mq_bc = mql3[:, tsl, :].to_broadcast([128, KNN - 1, KS])
    df = spool.tile([128, KNN - 1, KS], F32, tag=f"df{u}")
    nc.gpsimd.tensor_tensor(out=df[:, :, :], in0=gt[:, :, :], in1=mq_bc,
                            op=mybir.AluOpType.subtract)
    nc.vector.tensor_reduce(out=lt_all[:, tsl], in_=df[:, :, :],
                            axis=mybir.AxisListType.XY,
                            op=mybir.AluOpType.add,
                            apply_absolute_value=True)


def _body(tc, blob, loss_out, unroll=UNROLL):
    nc = tc.nc
    bap = blob.ap()
    mask_g = bap[OFF_MASK : OFF_MASK + W_MASK].bitcast(BF16).rearrange(
        "(n s) -> n s", s=KS)
    pc_all = [
        bap[OFF_PC + b * W_PC : OFF_PC + (b + 1) * W_PC]
        .bitcast(F16).rearrange("(d n) -> d n", d=3)
        for b in range(B)
    ]
    hl_all = [
        bap[OFF_HL + b * W_HL : OFF_HL + (b + 1) * W_HL]
        .bitcast(F16).rearrange("(d n) -> d n", d=2)
        for b in range(B)
    ]
    mask_qT = [
        bap[b * N * KS // 2 : (b + 1) * N * KS // 2].bitcast(BF16).rearrange(
            "(t p s) -> p t s", p=128, s=KS)
        for b in range(B)
    ]
    bias_g = bap[OFF_BIAS : OFF_BIAS + W_BIAS].bitcast(F32).rearrange(
        "(t p) -> p t", p=128)

    import contextlib
    with contextlib.ExitStack() as ctx:
        cpool = ctx.enter_context(tc.tile_pool(name="const", bufs=1))
        spool = ctx.enter_context(tc.tile_pool(name="small", bufs=3))
        ppool = ctx.enter_context(tc.tile_pool(name="psum", bufs=2, space="PSUM"))

        keyA = cpool.tile([128, W], F32)
        keyB = cpool.tile([128, W], F32)
        Cp16 = cpool.tile([5, B * N], F16)
        Qs16 = cpool.tile([5, B * N], F16)
        mql = cpool.tile([128, B * NT * KS], BF16)
        fioall = cpool.tile([128, B * NT * 8], U32)
        nsbias = cpool.tile([128, B * NT], F32)
        lt_all = cpool.tile([128, B * NT], F32)
        acc = cpool.tile([128, 1], F32)
        nc.vector.memset(Qs16[0:5, :], 1.0)
        for key in (keyA, keyB):
            nc.gpsimd.iota(_lo_lanes(key), [[1, W]], base=0,
                           channel_multiplier=0)
        nc.sync.dma_start(out=nsbias[:, :], in_=bias_g)
        for b in range(B):
            nc.sync.dma_start(out=Cp16[0:3, b * N : (b + 1) * N], in_=pc_all[b])
            nc.sync.dma_start(out=Cp16[3:5, b * N : (b + 1) * N], in_=hl_all[b])
            nc.sync.dma_start(
                out=mql.rearrange("p (t s) -> p t s", s=KS)[:, b * NT : (b + 1) * NT, :],
                in_=mask_qT[b])
        nc.scalar.mul(Qs16[0:3, :], Cp16[0:3, :], 2.0)

        Qs3 = Qs16.rearrange("d (t p) -> d t p", p=128)   # [5, B*NT, 128]
        mql3 = mql.rearrange("p (t s) -> p t s", s=KS)
        fio3 = fioall.rearrange("p (t e) -> p t e", e=8)

        # Fully static software-pipelined emission: no For_i, no all-engine
        # barriers — per-tile dependencies flow through Tile semaphores only.
        # Gathers for tile g issue right after its fio is final; the L1
        # consume lags LAG tiles so the gather DMAs are long done.
        LAG = 3
        keys = (keyA, keyB)
        windows = _CACHE["windows"]
        pend = []   # (global_tile, b, gt, slot)
        gslot = 0
        for b in range(B):
            for t in range(NT):
                _compute_tile(nc, t, b, spool, ppool, keys[t % 2], Cp16,
                              fio3,
                              lhsT=Qs3[:, b * NT + t : b * NT + t + 1, :].squeeze(1),
                              nbias=nsbias[:, b * NT + t : b * NT + t + 1],
                              lo_t=windows[b * NT + t])
                gt = _gather_issue(
                    nc, spool, mask_g,
                    fio3[:, t + b * NT : t + b * NT + 1, :].squeeze(1),
                    gslot % (LAG + 1))
                pend.append((t, b, gt))
                gslot += 1
                if len(pend) > LAG:
                    pt, pb, pgt = pend.pop(0)
                    _gather_consume(nc, pt, pb, spool, mql3, pgt, lt_all,
                                    gslot % (LAG + 1))
        for pt, pb, pgt in pend:
            gslot += 1
            _gather_consume(nc, pt, pb, spool, mql3, pgt, lt_all,
                            gslot % (LAG + 1))

        nc.vector.tensor_reduce(out=acc[:, :], in_=lt_all[:, :],
                                axis=mybir.AxisListType.X,
                                op=mybir.AluOpType.add)
        nc.sync.dma_start(out=loss_out.ap()[:, :], in_=acc[:, :])


def build_nc(unroll=UNROLL):
    nc = bacc.Bacc("TRN2", target_bir_lowering=False, debug=False,
                   num_devices=NCORES, enable_partition_id=False)
    blob = nc.dram_tensor("blob", [BLOB_LEN], U32, kind="ExternalInput")
    loss_out = nc.dram_tensor("loss_out", [128, 1], F32, kind="ExternalOutput")
    with tile.TileContext(nc) as tc:
        _body(tc, blob, loss_out, unroll=unroll)
    nc.compile()
    return nc


def make_in_maps(pc, mask):
    import ml_dtypes
    pc = np.asarray(np.asarray(pc), np.float32)
    mask = np.asarray(np.asarray(mask), np.float32)
    # sort each batch by z so every tile's in-radius candidates fall in a
    # static window of W sorted rows (|dz| <= 0.1); row identity is
    # preserved so the gathered-mask loss is invariant to the permutation
    pcs = np.empty_like(pc)
    masks = np.empty_like(mask)
    windows = []
    for b in range(B):
        order = np.argsort(pc[b][:, 2], kind="stable")
        pcs[b] = pc[b][order]
        masks[b] = mask[b][order]
        z = pcs[b][:, 2]
        for t in range(NT):
            zq = z[t * 128 : (t + 1) * 128]
            lo_b = int(np.searchsorted(z, zq.min() - np.float32(0.1001)))
            hi_b = int(np.searchsorted(z, zq.max() + np.float32(0.1001),
                                       side="right"))
            assert hi_b - lo_b <= W, (t, lo_b, hi_b)
            lo_b = max(0, min(lo_b, N - W))
            windows.append(lo_b)
    _CACHE["windows"] = tuple(windows)
    pc16 = pcs.astype(np.float16)
    maskb = masks.astype(ml_dtypes.bfloat16)
    parts = [maskb.reshape(-1).view(np.uint32)]
    for b in range(B):
        parts.append(np.ascontiguousarray(pc16[b].T).reshape(-1).view(np.uint32))
    hlparts, biasparts = [], []
    for b in range(B):
        c32 = pc16[b].astype(np.float32)
        csq = (c32 * c32).sum(-1)                       # f32 |c|^2 of fp16 coords
        hi = csq.astype(np.float16)
        lo = (csq - hi.astype(np.float32)).astype(np.float16)
        hlparts.append(np.concatenate([-hi, -lo]).reshape(-1).view(np.uint32))
        # bias rows in [t, p] order: query index = t*128 + p
        bias = (np.float32(BIAS0) - np.float32(SCALE) * csq).astype(np.float32)
        biasparts.append(bias.reshape(NT, 128))
    parts.extend(hlparts)
    parts.append(np.concatenate(biasparts, axis=0).reshape(-1).view(np.uint32))
    return [{"blob": np.concatenate(parts)}]


def kernel(pc, mask):
    in_maps = make_in_maps(pc, mask)
    ck = ("nc", _CACHE["windows"])
    if ck not in _CACHE:
        _CACHE[ck] = build_nc()
    nc = _CACHE[ck]
    res = run_bass_kernel_spmd(nc, in_maps, list(range(NCORES)))
    total = 0.0
    for r in res.results:
        total += r["loss_out"].astype(np.float64).sum()
    return np.float32(total / (B * N * KNN))
